# revision 1
# baseline (speedup 1.0000x reference)
"""BERT-base (12-layer, B=8, S=512, D=768, H=12, F=3072) forward pass on 8
Trainium2 NeuronCores.

Strategy: data-parallel over batch (1 sequence per core, no collectives).
Per core, activations are kept feature-major xT[D, S] in SBUF so that:
  - all big matmuls run as lhsT[dK,128] @ rhs[dK,512] fp32r at full PE rate
  - LayerNorm statistics (sums over the feature dim = partition dim) are
    ones-vector matmuls on the PE
  - softmax runs in scoresT [k, q] layout: the attention mask is a
    per-partition exp() bias, and denominators fall out of a ones-augmented
    V column in the P@V matmul (M=65)
  - per-column broadcasts (mu, rstd, 1/denom) are rank-1 ones-matmuls
Weights are pre-transposed on the host; fp32 bits are fed to the PE as
float32r (full-rate, ~1e-4 relative rounding).
"""
import numpy as np

import concourse.bass as bass
import concourse.mybir as mybir
import concourse.tile as tile
from concourse import bass_utils
from concourse.masks import make_identity

AF = mybir.ActivationFunctionType
OP = mybir.AluOpType
F32 = mybir.dt.float32
F32R = mybir.dt.float32r
BF16 = mybir.dt.bfloat16
I32 = mybir.dt.int32

B, S, D, H, F, L, V = 8, 512, 768, 12, 3072, 12, 30522
DK = D // H
SCALE = 1.0 / float(np.sqrt(DK))
NT = D // 128      # 6 feature tiles
NTF = F // 128     # 24 ffn tiles
NST = S // 128     # 4 sequence tiles
NP = H // 2        # 6 head pairs

_NC_CACHE = None


# ---------------------------------------------------------------------------
# wait-slot legalization: walrus codegen allows only ONE sync-wait command on
# TPB instructions; hoist excess waits into standalone EventSemaphores.
def _legalize_waits(nc):
    skip = (mybir.InstEventSemaphore, mybir.InstNoOp)
    n = 0
    for fn in nc.m.functions:
        for blk in fn.blocks:
            out = []
            for inst in blk.instructions:
                si = inst.sync_info
                if si is not None and si.on_wait and not isinstance(inst, skip) \
                        and len(si.on_wait) > 1:
                    waits = list(si.on_wait)
                    for j, w in enumerate(waits[:-1]):
                        ev = mybir.InstEventSemaphore(
                            name=f"{inst.name}-lgw{j}", ins=[], outs=[],
                            sync_info=mybir.SyncInfo(on_wait=[w], on_update=[]),
                        )
                        ev.engine = inst.engine
                        out.append(ev)
                        n += 1
                    inst.sync_info = mybir.SyncInfo(
                        on_wait=[waits[-1]], on_update=list(si.on_update))
                out.append(inst)
            try:
                blk.instructions = out
            except Exception:
                blk.instructions.clear()
                blk.instructions.extend(out)
    return n


def _build_nc():
    nc = bass.Bass("TRN2", target_bir_lowering=False, debug=False,
                   enable_asserts=False, num_devices=8)

    # ---- DRAM I/O ---------------------------------------------------------
    d_ids = nc.dram_tensor("ids", [S, 1], I32, kind="ExternalInput")
    d_tti = nc.dram_tensor("tti", [S, 1], I32, kind="ExternalInput")
    d_mask = nc.dram_tensor("maskadd", [S], F32, kind="ExternalInput")
    d_wemb = nc.dram_tensor("wemb", [V, D], F32, kind="ExternalInput")
    d_pemb = nc.dram_tensor("pemb", [S, D], F32, kind="ExternalInput")
    d_temb = nc.dram_tensor("temb", [2, D], F32, kind="ExternalInput")
    d_eg = nc.dram_tensor("eg", [D], F32, kind="ExternalInput")
    d_eb = nc.dram_tensor("eb", [D], F32, kind="ExternalInput")
    d_wq = nc.dram_tensor("wq", [L, D, D], BF16, kind="ExternalInput")
    d_wk = nc.dram_tensor("wk", [L, D, D], BF16, kind="ExternalInput")
    d_wv = nc.dram_tensor("wv", [L, D, D], BF16, kind="ExternalInput")
    d_wo = nc.dram_tensor("wo", [L, D, D], BF16, kind="ExternalInput")
    d_w1 = nc.dram_tensor("w1", [L, D, F], BF16, kind="ExternalInput")
    d_w2 = nc.dram_tensor("w2", [L, F, D], BF16, kind="ExternalInput")
    d_bq = nc.dram_tensor("bq", [L, D], F32, kind="ExternalInput")
    d_bk = nc.dram_tensor("bk", [L, D], F32, kind="ExternalInput")
    d_bv = nc.dram_tensor("bv", [L, D], F32, kind="ExternalInput")
    d_bo = nc.dram_tensor("bo", [L, D], F32, kind="ExternalInput")
    d_b1 = nc.dram_tensor("b1", [L, F], F32, kind="ExternalInput")
    d_b2 = nc.dram_tensor("b2", [L, D], F32, kind="ExternalInput")
    d_ag = nc.dram_tensor("ag", [L, D], F32, kind="ExternalInput")
    d_ab = nc.dram_tensor("ab", [L, D], F32, kind="ExternalInput")
    d_fg = nc.dram_tensor("fg", [L, D], F32, kind="ExternalInput")
    d_fb = nc.dram_tensor("fb", [L, D], F32, kind="ExternalInput")
    d_ones = nc.dram_tensor("ones128", [128], F32, kind="ExternalInput")
    d_onesb = nc.dram_tensor("ones128b", [128], BF16, kind="ExternalInput")
    d_ones512 = nc.dram_tensor("ones512", [1, 512], F32, kind="ExternalInput")
    d_neg1 = nc.dram_tensor("neg1", [1, 128], F32, kind="ExternalInput")
    d_selA = nc.dram_tensor("selA", [1, 128], F32, kind="ExternalInput")
    d_selB = nc.dram_tensor("selB", [1, 128], F32, kind="ExternalInput")
    d_onesgb = nc.dram_tensor("onesgridb", [128, NST * H], BF16, kind="ExternalInput")
    d_out = nc.dram_tensor("out", [S, D], F32, kind="ExternalOutput")

    with tile.TileContext(nc) as tc:
        _emit(nc, tc, locals())
    _legalize_waits(nc)
    return nc


def _emit(nc, tc, d):
    import contextlib
    ctx = contextlib.ExitStack()
    with ctx:
        _emit_body(nc, tc, d, ctx)


def _emit_body(nc, tc, d, ctx):
    pool = ctx.enter_context(tc.tile_pool(name="persist", bufs=1))
    wpool = ctx.enter_context(tc.tile_pool(name="weights", bufs=3))
    wqkpool = ctx.enter_context(tc.tile_pool(name="wqk", bufs=8))
    ppool = ctx.enter_context(tc.tile_pool(name="params", bufs=2))
    epool = ctx.enter_context(tc.tile_pool(name="epool", bufs=8))
    hpool = ctx.enter_context(tc.tile_pool(name="hpool", bufs=4))
    spool = ctx.enter_context(tc.tile_pool(name="smalls", bufs=1))

    # ---- persistent constants --------------------------------------------
    ones_col = pool.tile([128, 1], F32R, name="ones_col")
    nc.sync.dma_start(ones_col[:], d["d_ones"].ap().rearrange("(p o) -> p o", o=1).bitcast(F32R))
    ones_colb = pool.tile([128, 1], BF16, name="ones_colb")
    nc.sync.dma_start(ones_colb[:], d["d_onesb"].ap().rearrange("(p o) -> p o", o=1))
    one_row = pool.tile([1, 128], F32R, name="one_row")
    nc.sync.dma_start(one_row[:], d["d_ones"].ap().rearrange("(o p) -> o p", o=1).bitcast(F32R))
    ones_s = pool.tile([1, 512], F32R, name="ones_s")
    nc.sync.dma_start(ones_s[:], d["d_ones512"].ap()[:, :].bitcast(F32R))
    neg_row = pool.tile([1, 128], F32R, name="neg_row")
    nc.sync.dma_start(neg_row[:], d["d_neg1"].ap()[:, :].bitcast(F32R))
    selA = pool.tile([1, 128], F32R, name="selA")
    nc.sync.dma_start(selA[:], d["d_selA"].ap()[:, :].bitcast(F32R))
    selB = pool.tile([1, 128], F32R, name="selB")
    nc.sync.dma_start(selB[:], d["d_selB"].ap()[:, :].bitcast(F32R))
    ident = pool.tile([128, 128], F32, name="ident")
    make_identity(nc, ident[:])
    ident16 = pool.tile([128, 128], BF16, name="ident16")
    make_identity(nc, ident16[:])
    maskc = pool.tile([128, NST], F32, name="maskc")
    nc.sync.dma_start(maskc[:], d["d_mask"].ap().rearrange("(n p) -> p n", p=128))

    # ---- persistent activations ------------------------------------------
    xT = pool.tile([128, NT, S], BF16, name="xT")       # layer input, feature-major
    aT = pool.tile([128, NT, S], BF16, name="aT")       # post-attn LN out
    qT = pool.tile([128, NT, S], BF16, name="qT")
    kT = pool.tile([128, NT, S], BF16, name="kT")
    cT = pool.tile([128, NT, S], BF16, name="cT")       # ctx, feature-major
    ybuf = pool.tile([128, NT, S], F32R, name="ybuf")   # pre-LN staging
    vaug = pool.tile([128, NST, H, DK + 1], BF16, name="vaug")
    # ones column of vaug (written once)
    nc.sync.dma_start(
        vaug[:, :, :, DK:DK + 1],
        d["d_onesgb"].ap().rearrange("p (a b) -> p a b", a=NST)[:, :, :],
    )

    # =======================================================================
    # layernorm along the feature (partition-spread) dim, feature-major.
    # y: [128, nt, S] F32R tile; writes out[:, dt, :] F32R.
    def layernorm(y, nt, g_col, b_col, eps, out, psum_pool, dim):
        sq_t = []
        for dt in range(nt):
            sqt = spool.tile([128, S], BF16, name=f"sq{dt}", tag=f"sq{dt % 2}")
            nc.scalar.activation(sqt[:], y[:, dt, :].bitcast(F32), AF.Square)
            sq_t.append(sqt)
        s0 = psum_pool.tile([1, S], F32, name="s0", tag="st0")
        s1 = psum_pool.tile([1, S], F32, name="s1", tag="st1")
        for dt in range(nt):
            nc.tensor.matmul(s0[:], ones_col[:], y[:, dt, :],
                             start=(dt == 0), stop=(dt == nt - 1))
        for dt in range(nt):
            nc.tensor.matmul(s1[:], ones_colb[:], sq_t[dt][:],
                             start=(dt == 0), stop=(dt == nt - 1))
        mu = spool.tile([1, S], F32R, name="mu", tag="ln_mu")
        nc.vector.tensor_scalar(mu[:], s0[:], 1.0 / dim, None, OP.mult)
        msq = spool.tile([1, S], F32, name="msq", tag="ln_msq")
        nc.vector.tensor_scalar(msq[:], s1[:], 1.0 / dim, eps, OP.mult, OP.add)
        musq = spool.tile([1, S], F32, name="musq", tag="ln_musq")
        nc.vector.tensor_tensor(musq[:], mu[:].bitcast(F32), mu[:].bitcast(F32), op=OP.mult)
        var = spool.tile([1, S], F32R, name="var", tag="ln_var")
        nc.vector.tensor_tensor(var[:], msq[:], musq[:], op=OP.subtract)
        warm1 = psum_pool.tile([128, S], F32, name="warm1", tag="warm")
        nc.tensor.matmul(warm1[:], one_row[:], var[:], start=True, stop=True)
        lnv = spool.tile([1, S], F32, name="lnv", tag="ln_lnv")
        nc.scalar.activation(lnv[:], var[:].bitcast(F32), AF.Ln)
        rstd = spool.tile([1, S], F32R, name="rstd", tag="ln_rstd")
        nc.scalar.activation(rstd[:], lnv[:], AF.Exp, scale=-0.5)
        negmu_ps = psum_pool.tile([128, S], F32, name="negmu_ps", tag="bc0")
        nc.tensor.matmul(negmu_ps[:], neg_row[:], mu[:], start=True, stop=True)
        # pass 1 (in-place, overlaps the rstd chain): y -= mu
        nc.vector.tensor_tensor(y[:, 0, :], y[:, 0, :].bitcast(F32),
                                negmu_ps[:], op=OP.add)
        rstd_ps = psum_pool.tile([128, S], F32, name="rstd_ps", tag="bc1")
        nc.tensor.matmul(rstd_ps[:], one_row[:], rstd[:], start=True, stop=True)
        for dt in range(nt):
            nc.vector.tensor_tensor(out[:, dt, :], y[:, dt, :].bitcast(F32),
                                    rstd_ps[:], op=OP.mult)
            nc.scalar.activation(out[:, dt, :], out[:, dt, :],
                                 AF.Identity, bias=b_col[:, dt:dt + 1],
                                 scale=g_col[:, dt:dt + 1])
            if dt + 1 < nt:
                nc.vector.tensor_tensor(y[:, dt + 1, :], y[:, dt + 1, :].bitcast(F32),
                                        negmu_ps[:], op=OP.add)

    # =======================================================================
    # embedding: gather + add + transpose to feature-major + LN -> xT
    with (
        tc.tile_pool(name="emb_sb", bufs=1) as embp,
        tc.tile_pool(name="emb_ps", bufs=3, space="PSUM") as embps,
    ):
        egc = ppool.tile([128, NT], F32, name="egc")
        nc.sync.dma_start(egc[:], d["d_eg"].ap().rearrange("(n p) -> p n", p=128))
        ebc = ppool.tile([128, NT], F32, name="ebc")
        nc.sync.dma_start(ebc[:], d["d_eb"].ap().rearrange("(n p) -> p n", p=128))
        for st in range(NST):
            idst = embp.tile([128, 1], I32, name="idst", tag="idst")
            nc.sync.dma_start(idst[:], d["d_ids"].ap()[st * 128:(st + 1) * 128, :])
            ttst = embp.tile([128, 1], I32, name="ttst", tag="ttst")
            nc.sync.dma_start(ttst[:], d["d_tti"].ap()[st * 128:(st + 1) * 128, :])
            x0 = embp.tile([128, D], F32, name="x0", tag="x0")
            nc.gpsimd.indirect_dma_start(
                out=x0[:], out_offset=None, in_=d["d_wemb"].ap(),
                in_offset=bass.IndirectOffsetOnAxis(ap=idst[:, :1], axis=0))
            tg = embp.tile([128, D], F32, name="tg", tag="tg")
            nc.gpsimd.indirect_dma_start(
                out=tg[:], out_offset=None, in_=d["d_temb"].ap(),
                in_offset=bass.IndirectOffsetOnAxis(ap=ttst[:, :1], axis=0))
            pg = embp.tile([128, D], F32, name="pg", tag="pg")
            nc.sync.dma_start(pg[:], d["d_pemb"].ap()[st * 128:(st + 1) * 128, :])
            nc.vector.tensor_tensor(x0[:], x0[:], tg[:], op=OP.add)
            nc.vector.tensor_tensor(x0[:], x0[:], pg[:], op=OP.add)
            for dt in range(NT):
                trp = embps.tile([128, 128], F32, name="trp", tag="trp")
                nc.tensor.transpose(trp[:], x0[:, dt * 128:(dt + 1) * 128], ident[:])
                nc.vector.tensor_copy(ybuf[:, dt, st * 128:(st + 1) * 128], trp[:])
        with tc.tile_pool(name="eln_ps", bufs=1, space="PSUM") as elnps:
            layernorm(ybuf, NT, egc[:], ebc[:], 1e-12, xT, elnps, D)

    # =======================================================================
    # transformer layers
    for l in range(L):
        # ---- per-layer params -------------------------------------------
        bqc = ppool.tile([128, NT], F32, name="bqc", tag="bqc")
        nc.sync.dma_start(bqc[:], d["d_bq"].ap()[l].rearrange("(n p) -> p n", p=128))
        bkc = ppool.tile([128, NT], F32, name="bkc", tag="bkc")
        nc.sync.dma_start(bkc[:], d["d_bk"].ap()[l].rearrange("(n p) -> p n", p=128))
        bvr = ppool.tile([1, D], F32R, name="bvr", tag="bvr")
        nc.sync.dma_start(bvr[:], d["d_bv"].ap()[l].rearrange("(o e) -> o e", o=1).bitcast(F32R))
        bor = ppool.tile([1, D], F32R, name="bor", tag="bor")
        nc.sync.dma_start(bor[:], d["d_bo"].ap()[l].rearrange("(o e) -> o e", o=1).bitcast(F32R))
        b1c = ppool.tile([128, NTF], F32, name="b1c", tag="b1c")
        nc.sync.dma_start(b1c[:], d["d_b1"].ap()[l].rearrange("(n p) -> p n", p=128))
        b2c = ppool.tile([128, NT], F32, name="b2c", tag="b2c")
        nc.sync.dma_start(b2c[:], d["d_b2"].ap()[l].rearrange("(n p) -> p n", p=128))
        agc = ppool.tile([128, NT], F32, name="agc", tag="agc")
        nc.sync.dma_start(agc[:], d["d_ag"].ap()[l].rearrange("(n p) -> p n", p=128))
        abc = ppool.tile([128, NT], F32, name="abc", tag="abc")
        nc.sync.dma_start(abc[:], d["d_ab"].ap()[l].rearrange("(n p) -> p n", p=128))
        fgc = ppool.tile([128, NT], F32, name="fgc", tag="fgc")
        nc.sync.dma_start(fgc[:], d["d_fg"].ap()[l].rearrange("(n p) -> p n", p=128))
        fbc = ppool.tile([128, NT], F32, name="fbc", tag="fbc")
        nc.sync.dma_start(fbc[:], d["d_fb"].ap()[l].rearrange("(n p) -> p n", p=128))

        # ---- phase 1: q, k (feature-major) and v (seq-major) -------------
        with tc.tile_pool(name="qkv_ps", bufs=3, space="PSUM") as qps:
            for (wd, bcol, dst) in ((d["d_wq"], bqc, qT), (d["d_wk"], bkc, kT)):
                for et in range(NT):
                    wt = wqkpool.tile([128, NT, 128], BF16, name=f"wqk{et}", tag="wqk")
                    for g in range(3):
                        nc.sync.dma_start(
                            wt[:, 2 * g:2 * g + 2, :],
                            wd.ap()[l, g * 256:(g + 1) * 256, et * 128:(et + 1) * 128]
                            .rearrange("(n p) e -> p n e", p=128))
                    acc = qps.tile([128, S], F32, name=f"qk{et}", tag="acc")
                    for dt in range(NT):
                        nc.tensor.matmul(acc[:], wt[:, dt, :], xT[:, dt, :],
                                         start=(dt == 0), stop=(dt == NT - 1))
                    nc.scalar.activation(dst[:, et, :], acc[:], AF.Identity,
                                         bias=bcol[:, et:et + 1])
            # v: out[s-tile, e-chunk 256], lhsT = xT slices, rhs = WvT chunk
            for c in range(3):
                wv = wpool.tile([128, NT, 256], BF16, name=f"wv{c}", tag="wv")
                for g in range(3):
                    nc.sync.dma_start(
                        wv[:, 2 * g:2 * g + 2, :],
                        d["d_wv"].ap()[l, g * 256:(g + 1) * 256, c * 256:(c + 1) * 256]
                        .rearrange("(n p) e -> p n e", p=128))
                for st in range(NST):
                    acc = qps.tile([128, 256], F32, name=f"v{st}", tag="acc")
                    for dt in range(NT):
                        nc.tensor.matmul(acc[:], xT[:, dt, st * 128:(st + 1) * 128],
                                         wv[:, dt, :],
                                         start=(dt == 0), stop=False)
                    # + bias (rank-1: ones[s] x bv[e-chunk])
                    nc.tensor.matmul(acc[:], one_row[:],
                                     bvr[:, c * 256:(c + 1) * 256],
                                     start=False, stop=True, skip_group_check=True)
                    # scatter into vaug [.., head, 0:64]
                    nc.vector.tensor_copy(
                        vaug[:, st, c * 4:(c + 1) * 4, 0:DK],
                        acc[:].rearrange("p (a b) -> p a b", a=4))

        # ---- phase 2: attention -----------------------------------------
        with (
            tc.tile_pool(name="sc_ps", bufs=3, space="PSUM") as scps,
            tc.tile_pool(name="ctx_ps", bufs=1, space="PSUM") as ctxps,
            tc.tile_pool(name="den_ps", bufs=1, space="PSUM") as denps,
            tc.tile_pool(name="rc_ps", bufs=1, space="PSUM") as rcps,
        ):
            e_tiles = [None] * NP

            def emit_scores(p):
                ets = []
                for kt in range(NST):
                    for hh in range(2):
                        h = 2 * p + hh
                        lo, hi = hh * 64, hh * 64 + 64
                        sc = scps.tile([128, S], F32, name=f"sc{kt}{hh}", tag="sc")
                        nc.tensor.matmul(
                            sc[:], kT[lo:hi, p, kt * 128:(kt + 1) * 128],
                            qT[lo:hi, p, :], start=True, stop=True)
                        et = epool.tile([128, S], BF16, name=f"e{kt}{hh}", tag="e")
                        nc.scalar.activation(et[:], sc[:], AF.Exp,
                                             bias=maskc[:, kt:kt + 1])
                        ets.append(et)
                e_tiles[p] = ets

            def emit_pv(p):
                ets = e_tiles[p]
                # denominators: ones-matmuls over the exp tiles
                dens = []
                for hh in range(2):
                    den = denps.tile([1, S], F32, name=f"d{hh}", tag=f"den{hh}")
                    for kt in range(NST):
                        nc.tensor.matmul(den[:], ones_colb[:], ets[kt * 2 + hh][:],
                                         start=(kt == 0), stop=(kt == NST - 1))
                    dens.append(den)
                # PV: M=64 per head, separate banks
                cpss = []
                for hh in range(2):
                    h = 2 * p + hh
                    cps = ctxps.tile([DK, S], F32, name=f"cps{hh}", tag=f"ctx{hh}")
                    for kt in range(NST):
                        nc.tensor.matmul(cps[:],
                                         vaug[:, kt, h, 0:DK],
                                         ets[kt * 2 + hh][:],
                                         start=(kt == 0), stop=(kt == NST - 1))
                    cpss.append(cps)
                recips = []
                for hh in range(2):
                    nlden = spool.tile([1, S], F32, name=f"nld{hh}", tag=f"nlden{hh}")
                    nc.scalar.activation(nlden[:], dens[hh][:], AF.Ln)
                    recip = spool.tile([1, S], F32R, name=f"rcp{hh}", tag=f"recip{hh}")
                    nc.scalar.activation(recip[:], nlden[:], AF.Exp, scale=-1.0)
                    recips.append(recip)
                rps = rcps.tile([128, S], F32, name="rps", tag="rc")
                nc.tensor.matmul(rps[:], selA[:], recips[0][:], start=True, stop=False)
                nc.tensor.matmul(rps[:], selB[:], recips[1][:], start=False, stop=True,
                                 skip_group_check=True)
                rsb = spool.tile([128, S], F32, name="rsb", tag="rsb")
                nc.vector.tensor_copy(rsb[:], rps[:])
                for hh in range(2):
                    lo = hh * 64
                    nc.vector.tensor_tensor(cT[lo:lo + DK, p, :], cpss[hh][:],
                                            rsb[lo:lo + DK, :], op=OP.mult)

            emit_scores(0)
            for p in range(1, NP):
                emit_scores(p)
                emit_pv(p - 1)
            emit_pv(NP - 1)

        # ---- phase 3: Wo + residual -> ybuf ------------------------------
        with tc.tile_pool(name="wo_ps", bufs=3, space="PSUM") as wops:
            for et in range(NT):
                wt = wpool.tile([128, NT, 128], BF16, name=f"wo{et}", tag="wo")
                for g in range(3):
                    nc.sync.dma_start(
                        wt[:, 2 * g:2 * g + 2, :],
                        d["d_wo"].ap()[l, g * 256:(g + 1) * 256, et * 128:(et + 1) * 128]
                        .rearrange("(n p) e -> p n e", p=128))
                acc = wops.tile([128, S], F32, name=f"o{et}", tag="acc")
                for dt in range(NT):
                    nc.tensor.matmul(acc[:], wt[:, dt, :], cT[:, dt, :],
                                     start=(dt == 0), stop=False)
                nc.tensor.matmul(acc[:], bor[:, et * 128:(et + 1) * 128],
                                 ones_s[:], start=False, stop=True,
                                 skip_group_check=True)
                nc.vector.tensor_tensor(ybuf[:, et, :], acc[:],
                                        xT[:, et, :], op=OP.add)

        # ---- LN1 -> aT ---------------------------------------------------
        with tc.tile_pool(name="ln1_ps", bufs=1, space="PSUM") as lnps:
            layernorm(ybuf, NT, agc[:], abc[:], 1e-5, aT, lnps, D)

        # ---- phase 4: FFN (W1 -> h, W2 accumulate into 6 yT banks) -------
        with (
            tc.tile_pool(name="y_ps", bufs=1, space="PSUM") as yps,
            tc.tile_pool(name="h_ps", bufs=2, space="PSUM") as hps,
        ):
            ytiles = [yps.tile([128, S], F32, name=f"yt{et}", tag=f"y{et}")
                      for et in range(NT)]
            h_sb = [None] * NTF

            def emit_h(f):
                c, fj = divmod(f, 2)
                if fj == 0:
                    w1t = wpool.tile([128, NT, 256], BF16, name=f"w1_{c}", tag="w1")
                    for g in range(3):
                        nc.sync.dma_start(
                            w1t[:, 2 * g:2 * g + 2, :],
                            d["d_w1"].ap()[l, g * 256:(g + 1) * 256, c * 256:(c + 1) * 256]
                            .rearrange("(n p) e -> p n e", p=128))
                    emit_h.w1t = w1t
                    w2t = wpool.tile([128, 2, D], BF16, name=f"w2_{c}", tag="w2")
                    for g in range(2):
                        nc.sync.dma_start(
                            w2t[:, g:g + 1, :],
                            d["d_w2"].ap()[l, c * 256 + g * 128:c * 256 + (g + 1) * 128, :]
                            .rearrange("(n p) e -> p n e", p=128))
                    emit_h.w2t = w2t
                hacc = hps.tile([128, S], F32, name=f"h{f}", tag="hacc")
                for dt in range(NT):
                    nc.tensor.matmul(hacc[:], emit_h.w1t[:, dt, fj * 128:(fj + 1) * 128],
                                     aT[:, dt, :], start=(dt == 0), stop=(dt == NT - 1))
                hs = hpool.tile([128, S], BF16, name=f"hs{f}", tag="hs")
                nc.scalar.activation(hs[:], hacc[:], AF.Identity, bias=b1c[:, f:f + 1])
                h_sb[f] = hs
                emit_h.w2ts[f] = emit_h.w2t

            def emit_y(f):
                fj = f % 2
                w2t = emit_h.w2ts[f]
                for et in range(NT):
                    nc.tensor.matmul(ytiles[et][:],
                                     w2t[:, fj, et * 128:(et + 1) * 128],
                                     h_sb[f][:], start=(f == 0), stop=(f == NTF - 1))
                h_sb[f] = None

            emit_h.w2ts = [None] * NTF
            emit_h(0)
            for f in range(1, NTF):
                emit_h(f)
                emit_y(f - 1)
            emit_y(NTF - 1)

            # epilogue: +b2 (per-partition) -> ybuf
            for et in range(NT):
                nc.vector.tensor_scalar(ybuf[:, et, :], ytiles[et][:],
                                        b2c[:, et:et + 1], None, OP.add)

        # ---- LN2 -> xT (next layer input) --------------------------------
        with tc.tile_pool(name="ln2_ps", bufs=1, space="PSUM") as lnps:
            layernorm(ybuf, NT, fgc[:], fbc[:], 1e-5, xT, lnps, D)

    # =======================================================================
    # output: transpose xT -> [S, D] and DMA out
    with (
        tc.tile_pool(name="out_sb", bufs=2) as outp,
        tc.tile_pool(name="out_ps", bufs=2, space="PSUM") as outps,
    ):
        for st in range(NST):
            ops_t = outps.tile([128, D], BF16, name="ops", tag="ops")
            for dt in range(NT):
                nc.tensor.transpose(ops_t[:, dt * 128:(dt + 1) * 128],
                                    xT[:, dt, st * 128:(st + 1) * 128],
                                    ident16[:])
            osb = outp.tile([128, D], F32, name="osb", tag="osb")
            nc.vector.tensor_copy(osb[:], ops_t[:])
            nc.sync.dma_start(d["d_out"].ap()[st * 128:(st + 1) * 128, :], osb[:])


# ---------------------------------------------------------------------------
def kernel(**inputs):
    global _NC_CACHE
    if _NC_CACHE is None:
        _NC_CACHE = _build_nc()
    nc = _NC_CACHE

    import ml_dtypes
    f32 = lambda a: np.ascontiguousarray(np.asarray(a), dtype=np.float32)
    bf = lambda a: np.ascontiguousarray(a.astype(ml_dtypes.bfloat16))
    Wq = f32(inputs["Wq"]) * SCALE
    bq = f32(inputs["bq"]) * SCALE
    shared = {
        "wemb": f32(inputs["word_emb"]),
        "pemb": f32(inputs["pos_emb"])[:S],
        "temb": f32(inputs["type_emb"]),
        "eg": f32(inputs["emb_ln_g"]), "eb": f32(inputs["emb_ln_b"]),
        "wq": bf(Wq.transpose(0, 2, 1)),
        "wk": bf(f32(inputs["Wk"]).transpose(0, 2, 1)),
        "wv": bf(f32(inputs["Wv"]).transpose(0, 2, 1)),
        "wo": bf(f32(inputs["Wo"]).transpose(0, 2, 1)),
        "w1": bf(f32(inputs["W1"]).transpose(0, 2, 1)),
        "w2": bf(f32(inputs["W2"]).transpose(0, 2, 1)),
        "bq": bq, "bk": f32(inputs["bk"]), "bv": f32(inputs["bv"]),
        "bo": f32(inputs["bo"]), "b1": f32(inputs["b1"]), "b2": f32(inputs["b2"]),
        "ag": f32(inputs["attn_ln_g"]), "ab": f32(inputs["attn_ln_b"]),
        "fg": f32(inputs["ffn_ln_g"]), "fb": f32(inputs["ffn_ln_b"]),
        "ones128": np.ones(128, np.float32),
        "ones128b": np.ones(128, ml_dtypes.bfloat16),
        "ones512": np.ones((1, 512), np.float32),
        "neg1": np.full((1, 128), -1.0, np.float32),
        "selA": np.concatenate([np.ones((1, 64)), np.zeros((1, 64))], 1).astype(np.float32),
        "selB": np.concatenate([np.zeros((1, 64)), np.ones((1, 64))], 1).astype(np.float32),
        "onesgridb": np.ones((128, NST * H), ml_dtypes.bfloat16),
    }
    ids = np.asarray(inputs["input_ids"]).astype(np.int32)
    tti = np.asarray(inputs["token_type_ids"]).astype(np.int32)
    am = np.asarray(inputs["attention_mask"]).astype(np.float32)
    in_maps = []
    for c in range(B):
        in_maps.append({
            **shared,
            "ids": ids[c].reshape(S, 1),
            "tti": tti[c].reshape(S, 1),
            "maskadd": np.where(am[c] == 0, -1e9, 0.0).astype(np.float32),
        })
    res = bass_utils.run_bass_kernel_spmd(
        nc, in_maps, core_ids=list(range(B)), trace=False)
    out = np.stack([res.results[c]["out"] for c in range(B)], axis=0)
    return out.astype(np.float32)



# revision 2
# speedup vs baseline: 1.7413x; 1.7413x over previous
"""BERT-base (12-layer, B=8, S=512, D=768, H=12, F=3072) forward pass on 8
Trainium2 NeuronCores — v2.

Strategy: data-parallel over batch (1 sequence per core, no collectives).
Key structure (per core, activations feature-major xT[D, S] in SBUF):
  - the FFN in this model is LINEAR (no activation between W1/W2), so
    W2@W1 collapses on the host into one 768x768 matrix Weff; the attn-LN
    affine (g,b) folds into Weff/beff as well, and the LN normalize
    (mean/rstd) folds into the GEMM via a rank-1 correction + per-column
    scale at PSUM evacuation. The FFN costs 1/4 of the naive FLOPs and
    layer-norm #1 never stalls the PE.
  - softmax denominators fall out of a ones-augmented V column (PV matmul
    M=65); reciprocals via Ln/Exp on the denominator row (partition 64).
  - weights are host-repacked so every weight DMA is contiguous per
    partition (1.5-6KB lines instead of 256B).
  - emission interleaves Q/K projections with scores+exp per head-pair so
    the ACT-bound softmax overlaps PE GEMM work; dummy warm matmuls keep
    the PE HAM clock at 2.4GHz across LN chains.
"""
import numpy as np

import concourse.bass as bass
import concourse.mybir as mybir
import concourse.tile as tile
from concourse import bass_utils
from concourse.masks import make_identity

AF = mybir.ActivationFunctionType
OP = mybir.AluOpType
F32 = mybir.dt.float32
F32R = mybir.dt.float32r
BF16 = mybir.dt.bfloat16
I32 = mybir.dt.int32

B, S, D, H, F, L, V = 8, 512, 768, 12, 3072, 12, 30522
DK = D // H
SCALE = 1.0 / float(np.sqrt(DK))
NT = D // 128      # 6 feature tiles
NST = S // 128     # 4 sequence tiles
NP = H // 2        # 6 head pairs

_NC_CACHE = None


# ---------------------------------------------------------------------------
# wait-slot legalization: walrus codegen allows only ONE sync-wait command on
# TPB instructions; hoist excess waits into standalone EventSemaphores.
def _legalize_waits(nc):
    skip = (mybir.InstEventSemaphore, mybir.InstNoOp)
    n = 0
    for fn in nc.m.functions:
        for blk in fn.blocks:
            out = []
            for inst in blk.instructions:
                si = inst.sync_info
                if si is not None and si.on_wait and not isinstance(inst, skip) \
                        and len(si.on_wait) > 1:
                    waits = list(si.on_wait)
                    for j, w in enumerate(waits[:-1]):
                        ev = mybir.InstEventSemaphore(
                            name=f"{inst.name}-lgw{j}", ins=[], outs=[],
                            sync_info=mybir.SyncInfo(on_wait=[w], on_update=[]),
                        )
                        ev.engine = inst.engine
                        out.append(ev)
                        n += 1
                    inst.sync_info = mybir.SyncInfo(
                        on_wait=[waits[-1]], on_update=list(si.on_update))
                out.append(inst)
            try:
                blk.instructions = out
            except Exception:
                blk.instructions.clear()
                blk.instructions.extend(out)
    return n


def _build_nc():
    nc = bass.Bass("TRN2", target_bir_lowering=False, debug=False,
                   enable_asserts=False, num_devices=8)

    # ---- DRAM I/O ---------------------------------------------------------
    d = {}
    d["d_ids"] = nc.dram_tensor("ids", [S, 1], I32, kind="ExternalInput")
    d["d_tti"] = nc.dram_tensor("tti", [S, 1], I32, kind="ExternalInput")
    d["d_mask"] = nc.dram_tensor("maskadd", [S], F32, kind="ExternalInput")
    d["d_wemb"] = nc.dram_tensor("wemb", [V, D], F32, kind="ExternalInput")
    d["d_pemb"] = nc.dram_tensor("pemb", [S, D], F32, kind="ExternalInput")
    d["d_temb"] = nc.dram_tensor("temb", [2, D], F32, kind="ExternalInput")
    d["d_eg"] = nc.dram_tensor("eg", [D], F32, kind="ExternalInput")
    d["d_eb"] = nc.dram_tensor("eb", [D], F32, kind="ExternalInput")
    # repacked weights: [L, et, p, n, e] so each (l, et) chunk is contiguous
    d["d_wq"] = nc.dram_tensor("wq", [L, NT, 128, NT, 128], BF16, kind="ExternalInput")
    d["d_wk"] = nc.dram_tensor("wk", [L, NT, 128, NT, 128], BF16, kind="ExternalInput")
    d["d_wo"] = nc.dram_tensor("wo", [L, NT, 128, NT, 128], BF16, kind="ExternalInput")
    d["d_wf"] = nc.dram_tensor("wf", [L, NT, 128, NT, 128], F32, kind="ExternalInput")
    d["d_wva"] = nc.dram_tensor("wva", [L, 128, NT, 512], BF16, kind="ExternalInput")
    d["d_wvb"] = nc.dram_tensor("wvb", [L, 128, NT, 256], BF16, kind="ExternalInput")
    # packed per-layer params: columns [128, 30] and rows [1, 3*768]
    d["d_cols"] = nc.dram_tensor("cols", [L, 128, 30], F32, kind="ExternalInput")
    d["d_rows"] = nc.dram_tensor("rows", [L, 3 * D], F32, kind="ExternalInput")
    d["d_ones"] = nc.dram_tensor("ones128", [128], F32, kind="ExternalInput")
    d["d_onesb"] = nc.dram_tensor("ones128b", [128], BF16, kind="ExternalInput")
    d["d_ones512"] = nc.dram_tensor("ones512", [1, 512], F32, kind="ExternalInput")
    d["d_neg1"] = nc.dram_tensor("neg1", [1, 128], F32, kind="ExternalInput")
    d["d_selA"] = nc.dram_tensor("selA", [1, 128], F32, kind="ExternalInput")
    d["d_selB"] = nc.dram_tensor("selB", [1, 128], F32, kind="ExternalInput")
    d["d_onesgb"] = nc.dram_tensor("onesgridb", [128, NST * H], BF16, kind="ExternalInput")
    d["d_out"] = nc.dram_tensor("out", [S, D], F32, kind="ExternalOutput")

    with tile.TileContext(nc) as tc:
        _emit(nc, tc, d)
    _legalize_waits(nc)
    return nc


def _emit(nc, tc, d):
    import contextlib
    ctx = contextlib.ExitStack()
    with ctx:
        _emit_body(nc, tc, d, ctx)


def _emit_body(nc, tc, d, ctx):
    pool = ctx.enter_context(tc.tile_pool(name="persist", bufs=1))
    wqp = ctx.enter_context(tc.tile_pool(name="wqp", bufs=2))
    wkp = ctx.enter_context(tc.tile_pool(name="wkp", bufs=2))
    wop = ctx.enter_context(tc.tile_pool(name="wop", bufs=6))
    wfp = ctx.enter_context(tc.tile_pool(name="wfp", bufs=6))
    vwp = ctx.enter_context(tc.tile_pool(name="vwp", bufs=2))
    ppool = ctx.enter_context(tc.tile_pool(name="params", bufs=2))
    epool = ctx.enter_context(tc.tile_pool(name="epool", bufs=6))
    spool = ctx.enter_context(tc.tile_pool(name="smalls", bufs=1))
    sqpool = ctx.enter_context(tc.tile_pool(name="sqp", bufs=1))

    # ---- persistent constants --------------------------------------------
    ones_col = pool.tile([128, 1], F32R, name="ones_col")
    nc.sync.dma_start(ones_col[:], d["d_ones"].ap().rearrange("(p o) -> p o", o=1).bitcast(F32R))
    ones_colb = pool.tile([128, 1], BF16, name="ones_colb")
    nc.sync.dma_start(ones_colb[:], d["d_onesb"].ap().rearrange("(p o) -> p o", o=1))
    one_row = pool.tile([1, 128], F32R, name="one_row")
    nc.sync.dma_start(one_row[:], d["d_ones"].ap().rearrange("(o p) -> o p", o=1).bitcast(F32R))
    ones_s = pool.tile([1, 512], F32R, name="ones_s")
    nc.sync.dma_start(ones_s[:], d["d_ones512"].ap()[:, :].bitcast(F32R))
    neg_row = pool.tile([1, 128], F32R, name="neg_row")
    nc.sync.dma_start(neg_row[:], d["d_neg1"].ap()[:, :].bitcast(F32R))
    # head-select rows living on partition 64 (same partition as the PV
    # denominator row) so the broadcast matmul's operands share a base.
    sel64 = pool.tile([65, 2, 128], F32R, name="sel64")
    nc.sync.dma_start(sel64[64:65, 0, :], d["d_selA"].ap()[:, :].bitcast(F32R))
    nc.sync.dma_start(sel64[64:65, 1, :], d["d_selB"].ap()[:, :].bitcast(F32R))
    ident = pool.tile([128, 128], F32, name="ident")
    make_identity(nc, ident[:])
    ident16 = pool.tile([128, 128], BF16, name="ident16")
    make_identity(nc, ident16[:])
    maskc = pool.tile([128, NST], F32, name="maskc")
    nc.sync.dma_start(maskc[:], d["d_mask"].ap().rearrange("(n p) -> p n", p=128))

    # ---- persistent activations ------------------------------------------
    xT = pool.tile([128, NT, S], BF16, name="xT")       # layer input, feature-major
    qT = pool.tile([128, NT, S], BF16, name="qT")
    kT = pool.tile([128, NT, S], BF16, name="kT")
    cT = pool.tile([128, NT, S], BF16, name="cT")       # ctx, feature-major
    ybuf = pool.tile([128, NT, S], F32R, name="ybuf")   # post-Wo residual
    ybuf2 = pool.tile([128, NT, S], F32R, name="ybuf2")  # post-FFN (pre-LN2)
    vaug = pool.tile([128, NST, H, DK + 1], BF16, name="vaug")
    nc.sync.dma_start(
        vaug[:, :, :, DK:DK + 1],
        d["d_onesgb"].ap().rearrange("p (a b) -> p a b", a=NST)[:, :, :],
    )

    def warm_mm(wps):
        t = wps.tile([128, S], F32, name="warm", tag="warm")
        nc.tensor.matmul(t[:], one_row[:], ones_s[:], start=True, stop=True)

    # =======================================================================
    # standard layernorm (used for embedding LN): y [128, nt, S] F32R ->
    # out[:, dt, :] with per-partition affine (g_col, b_col).
    def layernorm(y, nt, g_col, b_col, eps, out, psum_pool, dim):
        sq_t = []
        for dt in range(nt):
            sqt = spool.tile([128, S], BF16, name=f"sq{dt}", tag=f"sq{dt % 2}")
            nc.vector.tensor_tensor(sqt[:], y[:, dt, :].bitcast(F32),
                                    y[:, dt, :].bitcast(F32), op=OP.mult)
            sq_t.append(sqt)
        s0 = psum_pool.tile([1, S], F32, name="s0", tag="st0")
        s1 = psum_pool.tile([1, S], F32, name="s1", tag="st1")
        for dt in range(nt):
            nc.tensor.matmul(s0[:], ones_col[:], y[:, dt, :],
                             start=(dt == 0), stop=(dt == nt - 1))
        for dt in range(nt):
            nc.tensor.matmul(s1[:], ones_colb[:], sq_t[dt][:],
                             start=(dt == 0), stop=(dt == nt - 1))
        mu = spool.tile([1, S], F32R, name="mu", tag="ln_mu")
        nc.vector.tensor_scalar(mu[:], s0[:], 1.0 / dim, None, OP.mult)
        msq = spool.tile([1, S], F32, name="msq", tag="ln_msq")
        nc.vector.tensor_scalar(msq[:], s1[:], 1.0 / dim, eps, OP.mult, OP.add)
        musq = spool.tile([1, S], F32, name="musq", tag="ln_musq")
        nc.vector.tensor_tensor(musq[:], mu[:].bitcast(F32), mu[:].bitcast(F32), op=OP.mult)
        var = spool.tile([1, S], F32R, name="var", tag="ln_var")
        nc.vector.tensor_tensor(var[:], msq[:], musq[:], op=OP.subtract)
        warm_mm(psum_pool)
        lnv = spool.tile([1, S], F32, name="lnv", tag="ln_lnv")
        nc.scalar.activation(lnv[:], var[:].bitcast(F32), AF.Ln)
        rstd = spool.tile([1, S], F32R, name="rstd", tag="ln_rstd")
        nc.scalar.activation(rstd[:], lnv[:], AF.Exp, scale=-0.5)
        negmu_ps = psum_pool.tile([128, S], F32, name="negmu_ps", tag="bc0")
        nc.tensor.matmul(negmu_ps[:], neg_row[:], mu[:], start=True, stop=True)
        nc.vector.tensor_tensor(y[:, 0, :], y[:, 0, :].bitcast(F32),
                                negmu_ps[:], op=OP.add)
        rstd_ps = psum_pool.tile([128, S], F32, name="rstd_ps", tag="bc1")
        nc.tensor.matmul(rstd_ps[:], one_row[:], rstd[:], start=True, stop=True)
        for dt in range(nt):
            nc.vector.tensor_tensor(out[:, dt, :], y[:, dt, :].bitcast(F32),
                                    rstd_ps[:], op=OP.mult)
            nc.scalar.activation(out[:, dt, :], out[:, dt, :],
                                 AF.Identity, bias=b_col[:, dt:dt + 1],
                                 scale=g_col[:, dt:dt + 1])
            if dt == 2:
                warm_mm(psum_pool)
            if dt + 1 < nt:
                nc.vector.tensor_tensor(y[:, dt + 1, :], y[:, dt + 1, :].bitcast(F32),
                                        negmu_ps[:], op=OP.add)

    # =======================================================================
    # embedding: gather + add + transpose to feature-major + LN -> xT
    with (
        tc.tile_pool(name="emb_sb", bufs=1) as embp,
        tc.tile_pool(name="emb_ps", bufs=3, space="PSUM") as embps,
    ):
        egc = ppool.tile([128, NT], F32, name="egc")
        nc.sync.dma_start(egc[:], d["d_eg"].ap().rearrange("(n p) -> p n", p=128))
        ebc = ppool.tile([128, NT], F32, name="ebc")
        nc.sync.dma_start(ebc[:], d["d_eb"].ap().rearrange("(n p) -> p n", p=128))
        for st in range(NST):
            idst = embp.tile([128, 1], I32, name="idst", tag="idst")
            nc.sync.dma_start(idst[:], d["d_ids"].ap()[st * 128:(st + 1) * 128, :])
            ttst = embp.tile([128, 1], I32, name="ttst", tag="ttst")
            nc.sync.dma_start(ttst[:], d["d_tti"].ap()[st * 128:(st + 1) * 128, :])
            x0 = embp.tile([128, D], F32, name="x0", tag="x0")
            nc.gpsimd.indirect_dma_start(
                out=x0[:], out_offset=None, in_=d["d_wemb"].ap(),
                in_offset=bass.IndirectOffsetOnAxis(ap=idst[:, :1], axis=0))
            tg = embp.tile([128, D], F32, name="tg", tag="tg")
            nc.gpsimd.indirect_dma_start(
                out=tg[:], out_offset=None, in_=d["d_temb"].ap(),
                in_offset=bass.IndirectOffsetOnAxis(ap=ttst[:, :1], axis=0))
            pg = embp.tile([128, D], F32, name="pg", tag="pg")
            nc.sync.dma_start(pg[:], d["d_pemb"].ap()[st * 128:(st + 1) * 128, :])
            nc.vector.tensor_tensor(x0[:], x0[:], tg[:], op=OP.add)
            nc.vector.tensor_tensor(x0[:], x0[:], pg[:], op=OP.add)
            for dt in range(NT):
                trp = embps.tile([128, 128], F32, name="trp", tag="trp")
                nc.tensor.transpose(trp[:], x0[:, dt * 128:(dt + 1) * 128], ident[:])
                nc.vector.tensor_copy(ybuf[:, dt, st * 128:(st + 1) * 128], trp[:])
        with tc.tile_pool(name="eln_ps", bufs=1, space="PSUM") as elnps:
            layernorm(ybuf, NT, egc[:], ebc[:], 1e-12, xT, elnps, D)

    # =======================================================================
    # transformer layers
    for l in range(L):
        # ---- per-layer params (two packed DMAs) --------------------------
        colsc = ppool.tile([128, 30], F32, name="colsc", tag="colsc")
        nc.sync.dma_start(colsc[:], d["d_cols"].ap()[l])
        rows = ppool.tile([1, 3 * D], F32R, name="rows", tag="rows")
        nc.sync.dma_start(rows[:], d["d_rows"].ap()[l].rearrange("(o e) -> o e", o=1).bitcast(F32R))
        bqc = colsc[:, 0:6]
        bkc = colsc[:, 6:12]
        beffc = colsc[:, 12:18]
        fgc = colsc[:, 18:24]
        fbc = colsc[:, 24:30]
        bvr = rows[:, 0:D]
        bor = rows[:, D:2 * D]
        wesum = rows[:, 2 * D:3 * D]

        # ---- attention-scope psum pools ----------------------------------
        with (
            tc.tile_pool(name="accp", bufs=3, space="PSUM") as accp,
            tc.tile_pool(name="scp", bufs=1, space="PSUM") as scp,
            tc.tile_pool(name="ctxp", bufs=1, space="PSUM") as ctxp,
            tc.tile_pool(name="rcp", bufs=1, space="PSUM") as rcp,
        ):
            # ---- V (seq-major, two column halves) ------------------------
            wva = vwp.tile([128, NT, 512], BF16, name="wva", tag="va")
            nc.sync.dma_start(wva[:], d["d_wva"].ap()[l])
            wvb = vwp.tile([128, NT, 256], BF16, name="wvb", tag="vb")
            nc.sync.dma_start(wvb[:], d["d_wvb"].ap()[l])
            for st in range(NST):
                acc = accp.tile([128, 512], F32, name=f"va{st}", tag="acc")
                for dt in range(NT):
                    nc.tensor.matmul(acc[:], xT[:, dt, st * 128:(st + 1) * 128],
                                     wva[:, dt, :], start=(dt == 0), stop=False)
                nc.tensor.matmul(acc[:], one_row[:], bvr[0:1, 0:512],
                                 start=False, stop=True, skip_group_check=True)
                nc.vector.tensor_copy(
                    vaug[:, st, 0:8, 0:DK],
                    acc[:].rearrange("p (a b) -> p a b", a=8))
            for st in range(NST):
                acc = accp.tile([128, 256], F32, name=f"vb{st}", tag="acc")
                for dt in range(NT):
                    nc.tensor.matmul(acc[:], xT[:, dt, st * 128:(st + 1) * 128],
                                     wvb[:, dt, :], start=(dt == 0), stop=False)
                nc.tensor.matmul(acc[:], one_row[:], bvr[0:1, 512:768],
                                 start=False, stop=True, skip_group_check=True)
                nc.vector.tensor_copy(
                    vaug[:, st, 8:12, 0:DK],
                    acc[:].rearrange("p (a b) -> p a b", a=4))

            # ---- attention machinery -------------------------------------
            e_tiles = [None] * NP

            def emit_qk(p):
                for (wpool_, wd, bcol, dst, nm) in (
                        (wqp, d["d_wq"], bqc, qT, "q"), (wkp, d["d_wk"], bkc, kT, "k")):
                    wt = wpool_.tile([128, NT, 128], BF16, name=f"w{nm}{p}", tag=f"w{nm}")
                    nc.sync.dma_start(wt[:], wd.ap()[l, p])
                    acc = accp.tile([128, S], F32, name=f"{nm}{p}", tag="acc")
                    for dt in range(NT):
                        nc.tensor.matmul(acc[:], wt[:, dt, :], xT[:, dt, :],
                                         start=(dt == 0), stop=(dt == NT - 1))
                    nc.vector.tensor_scalar(dst[:, p, :], acc[:],
                                            bcol[:, p:p + 1], None, OP.add)

            def emit_scores(p):
                ets = []
                for kt in range(NST):
                    sc = scp.tile([128, 2, S], F32, name=f"sc{kt}", tag="sc")
                    for hh in range(2):
                        lo, hi = hh * 64, hh * 64 + 64
                        nc.tensor.matmul(
                            sc[:, hh, :], kT[lo:hi, p, kt * 128:(kt + 1) * 128],
                            qT[lo:hi, p, :], start=True, stop=True)
                    et = epool.tile([128, 2, S], BF16, name=f"e{kt}", tag="e")
                    nc.scalar.activation(et[:, :, :], sc[:, :, :], AF.Exp,
                                         bias=maskc[:, kt:kt + 1])
                    ets.append(et)
                e_tiles[p] = ets

            def emit_pv(p):
                ets = e_tiles[p]
                cpss = []
                for hh in range(2):
                    h = 2 * p + hh
                    cps = ctxp.tile([DK + 1, S], F32, name=f"cps{hh}", tag=f"ctx{hh}")
                    for kt in range(NST):
                        nc.tensor.matmul(cps[:],
                                         vaug[:, kt, h, 0:DK + 1],
                                         ets[kt][:, hh, :],
                                         start=(kt == 0), stop=(kt == NST - 1))
                    cpss.append(cps)
                # reciprocal of the denominator row (partition 64)
                recips = []
                for hh in range(2):
                    nld = spool.tile([65, S], F32, name=f"nld{hh}", tag=f"nlden{hh}")
                    nc.scalar.activation(nld[64:65, :], cpss[hh][64:65, :], AF.Ln)
                    recip = spool.tile([65, S], F32R, name=f"rcp{hh}", tag=f"recip{hh}")
                    nc.scalar.activation(recip[64:65, :], nld[64:65, :], AF.Exp,
                                         scale=-1.0)
                    recips.append(recip)
                rps = rcp.tile([128, S], F32, name="rps", tag="rc")
                nc.tensor.matmul(rps[:], sel64[64:65, 0, :], recips[0][64:65, :],
                                 start=True, stop=False)
                nc.tensor.matmul(rps[:], sel64[64:65, 1, :], recips[1][64:65, :],
                                 start=False, stop=True, skip_group_check=True)
                rsb = spool.tile([128, S], F32, name="rsb", tag="rsb")
                nc.vector.tensor_copy(rsb[:], rps[:])
                for hh in range(2):
                    lo = hh * 64
                    nc.vector.tensor_tensor(cT[lo:lo + DK, p, :], cpss[hh][0:DK, :],
                                            rsb[lo:lo + DK, :], op=OP.mult)

            # pipeline: Q/K + scores run ahead; PV trails by 2 pairs
            for p in range(NP):
                emit_qk(p)
                emit_scores(p)
                if p >= 2:
                    emit_pv(p - 2)
            emit_pv(NP - 2)
            emit_pv(NP - 1)

            # ---- Wo + residual -> ybuf; squares chase for LN1 ------------
            sq1 = [None] * NT
            for et in range(NT):
                wt = wop.tile([128, NT, 128], BF16, name=f"wo{et}", tag="wo")
                nc.sync.dma_start(wt[:], d["d_wo"].ap()[l, et])
                acc = accp.tile([128, S], F32, name=f"o{et}", tag="acc")
                for dt in range(NT):
                    nc.tensor.matmul(acc[:], wt[:, dt, :], cT[:, dt, :],
                                     start=(dt == 0), stop=False)
                nc.tensor.matmul(acc[:], bor[0:1, et * 128:(et + 1) * 128],
                                 ones_s[:], start=False, stop=True,
                                 skip_group_check=True)
                nc.vector.tensor_tensor(ybuf[:, et, :], acc[:],
                                        xT[:, et, :], op=OP.add)
                sqt = sqpool.tile([128, S], BF16, name=f"sq1_{et}", tag=f"sq1_{et}")
                nc.vector.tensor_tensor(sqt[:], ybuf[:, et, :].bitcast(F32),
                                        ybuf[:, et, :].bitcast(F32), op=OP.mult)
                sq1[et] = sqt

        # ---- LN1 (folded into Weff) + FFN --------------------------------
        with (
            tc.tile_pool(name="lnp", bufs=1, space="PSUM") as lnp,
            tc.tile_pool(name="yaccp", bufs=3, space="PSUM") as yaccp,
        ):
            s0 = lnp.tile([1, S], F32, name="s0", tag="st0")
            s1 = lnp.tile([1, S], F32, name="s1", tag="st1")
            for dt in range(NT):
                nc.tensor.matmul(s0[:], ones_col[:], ybuf[:, dt, :],
                                 start=(dt == 0), stop=(dt == NT - 1))
            for dt in range(NT):
                nc.tensor.matmul(s1[:], ones_colb[:], sq1[dt][:],
                                 start=(dt == 0), stop=(dt == NT - 1))
            negmu1 = spool.tile([1, S], F32R, name="negmu1", tag="negmu1")
            nc.vector.tensor_scalar(negmu1[:], s0[:], -1.0 / D, None, OP.mult)
            msq = spool.tile([1, S], F32, name="msq1", tag="ln_msq")
            nc.vector.tensor_scalar(msq[:], s1[:], 1.0 / D, 1e-5, OP.mult, OP.add)
            musq = spool.tile([1, S], F32, name="musq1", tag="ln_musq")
            nc.vector.tensor_tensor(musq[:], negmu1[:].bitcast(F32),
                                    negmu1[:].bitcast(F32), op=OP.mult)
            var = spool.tile([1, S], F32, name="var1", tag="ln_var")
            nc.vector.tensor_tensor(var[:], msq[:], musq[:], op=OP.subtract)
            lnv = spool.tile([1, S], F32, name="lnv1", tag="ln_lnv")
            nc.scalar.activation(lnv[:], var[:], AF.Ln)
            rstd1 = spool.tile([1, S], F32R, name="rstd1", tag="ln_rstd")
            nc.scalar.activation(rstd1[:], lnv[:], AF.Exp, scale=-0.5)

            # FFN: y2 = rstd1 .col* (Weffg @ ybuf - mu1 x wesum) + beff
            rstd1b = spool.tile([128, S], F32, name="rstd1b", tag="rstd1b")
            sq2 = [None] * NT

            def emit_ffn_group(et):
                wt = wfp.tile([128, NT, 128], F32R, name=f"wf{et}", tag="wf")
                nc.sync.dma_start(wt[:], d["d_wf"].ap()[l, et].bitcast(F32R))
                acc = yaccp.tile([128, S], F32, name=f"y{et}", tag="yacc")
                for dt in range(NT):
                    nc.tensor.matmul(acc[:], wt[:, dt, :], ybuf[:, dt, :],
                                     start=(dt == 0), stop=False)
                nc.tensor.matmul(acc[:], wesum[0:1, et * 128:(et + 1) * 128],
                                 negmu1[:], start=False, stop=True,
                                 skip_group_check=True)
                return acc

            def emit_ffn_evac(et, acc):
                nc.vector.tensor_tensor(ybuf2[:, et, :], acc[:], rstd1b[:],
                                        op=OP.mult)
                nc.scalar.activation(ybuf2[:, et, :], ybuf2[:, et, :],
                                     AF.Identity, bias=beffc[:, et:et + 1])
                sqt = sqpool.tile([128, S], BF16, name=f"sq2_{et}", tag=f"sq2_{et}")
                nc.vector.tensor_tensor(sqt[:], ybuf2[:, et, :].bitcast(F32),
                                        ybuf2[:, et, :].bitcast(F32), op=OP.mult)
                sq2[et] = sqt

            accs = [None] * NT
            accs[0] = emit_ffn_group(0)
            accs[1] = emit_ffn_group(1)
            # rstd broadcast after two groups: the ACT chain has finished by
            # then, so the PE never stalls; evacs unblock before group 3
            # needs a psum slot.
            rstd1_ps = lnp.tile([128, S], F32, name="rstd1_ps", tag="bc1")
            nc.tensor.matmul(rstd1_ps[:], one_row[:], rstd1[:], start=True, stop=True)
            nc.vector.tensor_copy(rstd1b[:], rstd1_ps[:])
            emit_ffn_evac(0, accs[0])
            for et in range(2, NT):
                accs[et] = emit_ffn_group(et)
                emit_ffn_evac(et - 1, accs[et - 1])
            emit_ffn_evac(NT - 1, accs[NT - 1])

        # ---- LN2 -> xT (next layer input) --------------------------------
        with tc.tile_pool(name="ln2p", bufs=1, space="PSUM") as ln2p:
            s0 = ln2p.tile([1, S], F32, name="s0", tag="st0")
            s1 = ln2p.tile([1, S], F32, name="s1", tag="st1")
            for dt in range(NT):
                nc.tensor.matmul(s0[:], ones_col[:], ybuf2[:, dt, :],
                                 start=(dt == 0), stop=(dt == NT - 1))
            for dt in range(NT):
                nc.tensor.matmul(s1[:], ones_colb[:], sq2[dt][:],
                                 start=(dt == 0), stop=(dt == NT - 1))
            mu = spool.tile([1, S], F32R, name="mu2", tag="ln_mu")
            nc.vector.tensor_scalar(mu[:], s0[:], 1.0 / D, None, OP.mult)
            msq = spool.tile([1, S], F32, name="msq2", tag="ln_msq")
            nc.vector.tensor_scalar(msq[:], s1[:], 1.0 / D, 1e-5, OP.mult, OP.add)
            musq = spool.tile([1, S], F32, name="musq2", tag="ln_musq")
            nc.vector.tensor_tensor(musq[:], mu[:].bitcast(F32), mu[:].bitcast(F32),
                                    op=OP.mult)
            var = spool.tile([1, S], F32, name="var2", tag="ln_var")
            nc.vector.tensor_tensor(var[:], msq[:], musq[:], op=OP.subtract)
            warm_mm(ln2p)
            lnv = spool.tile([1, S], F32, name="lnv2", tag="ln_lnv")
            nc.scalar.activation(lnv[:], var[:], AF.Ln)
            rstd = spool.tile([1, S], F32R, name="rstd2", tag="ln_rstd")
            nc.scalar.activation(rstd[:], lnv[:], AF.Exp, scale=-0.5)
            negmu_ps = ln2p.tile([128, S], F32, name="negmu_ps", tag="bc0")
            nc.tensor.matmul(negmu_ps[:], neg_row[:], mu[:], start=True, stop=True)
            nc.vector.tensor_tensor(ybuf2[:, 0, :], ybuf2[:, 0, :].bitcast(F32),
                                    negmu_ps[:], op=OP.add)
            rstd_ps = ln2p.tile([128, S], F32, name="rstd_ps", tag="bc1")
            nc.tensor.matmul(rstd_ps[:], one_row[:], rstd[:], start=True, stop=True)
            for dt in range(NT):
                nc.vector.tensor_tensor(xT[:, dt, :], ybuf2[:, dt, :].bitcast(F32),
                                        rstd_ps[:], op=OP.mult)
                nc.scalar.activation(xT[:, dt, :], xT[:, dt, :],
                                     AF.Identity, bias=fbc[:, dt:dt + 1],
                                     scale=fgc[:, dt:dt + 1])
                if dt == 2:
                    warm_mm(ln2p)
                if dt + 1 < NT:
                    nc.vector.tensor_tensor(ybuf2[:, dt + 1, :],
                                            ybuf2[:, dt + 1, :].bitcast(F32),
                                            negmu_ps[:], op=OP.add)

    # =======================================================================
    # output: transpose xT -> [S, D] and DMA out
    with (
        tc.tile_pool(name="out_sb", bufs=2) as outp,
        tc.tile_pool(name="out_ps", bufs=2, space="PSUM") as outps,
    ):
        for st in range(NST):
            ops_t = outps.tile([128, D], BF16, name="ops", tag="ops")
            for dt in range(NT):
                nc.tensor.transpose(ops_t[:, dt * 128:(dt + 1) * 128],
                                    xT[:, dt, st * 128:(st + 1) * 128],
                                    ident16[:])
            osb = outp.tile([128, D], F32, name="osb", tag="osb")
            nc.vector.tensor_copy(osb[:], ops_t[:])
            nc.sync.dma_start(d["d_out"].ap()[st * 128:(st + 1) * 128, :], osb[:])


# ---------------------------------------------------------------------------
def _pack_dd(w):
    """[L, Din, Dout] -> [L, et, p, n, e] with w[l, n*128+p, et*128+e]."""
    Lw = w.shape[0]
    return np.ascontiguousarray(
        w.reshape(Lw, NT, 128, NT, 128).transpose(0, 3, 2, 1, 4))


def kernel(**inputs):
    global _NC_CACHE
    if _NC_CACHE is None:
        _NC_CACHE = _build_nc()
    nc = _NC_CACHE

    import ml_dtypes
    f32 = lambda a: np.ascontiguousarray(np.asarray(a), dtype=np.float32)
    bf = lambda a: np.ascontiguousarray(a.astype(ml_dtypes.bfloat16))

    Wq = f32(inputs["Wq"]) * SCALE
    bq = f32(inputs["bq"]) * SCALE
    Wk = f32(inputs["Wk"])
    Wv = f32(inputs["Wv"])
    Wo = f32(inputs["Wo"])
    W1 = f32(inputs["W1"])
    W2 = f32(inputs["W2"])
    b1 = f32(inputs["b1"])
    b2 = f32(inputs["b2"])
    ag = f32(inputs["attn_ln_g"])
    ab = f32(inputs["attn_ln_b"])

    # collapse the (linear) FFN: y2 = W2 @ (W1 @ a + b1) + b2 with
    # a = g * z + b  =>  y2 = Weffg @ z + beff
    Weff = np.einsum("ldf,lfe->lde", W2, W1)          # [L, D, D]
    Weffg = Weff * ag[:, None, :]                      # fold LN1 gamma
    beff = (np.einsum("lde,le->ld", Weff, ab)
            + np.einsum("ldf,lf->ld", W2, b1) + b2)    # fold LN1 beta + biases
    wesum = Weffg.sum(axis=2)                          # [L, D]

    WqT = Wq.transpose(0, 2, 1)
    WkT = Wk.transpose(0, 2, 1)
    WvT = Wv.transpose(0, 2, 1)
    WoT = Wo.transpose(0, 2, 1)
    WfT = Weffg.transpose(0, 2, 1)

    WvA = WvT.reshape(L, NT, 128, D)                   # [l, n, p, e]
    wva = np.ascontiguousarray(WvA[:, :, :, 0:512].transpose(0, 2, 1, 3))
    wvb = np.ascontiguousarray(WvA[:, :, :, 512:768].transpose(0, 2, 1, 3))

    r = lambda a: a.reshape(L, NT, 128).transpose(0, 2, 1)  # [L,128,NT] cols
    cols = np.concatenate([r(bq), r(f32(inputs["bk"])), r(beff),
                           r(f32(inputs["ffn_ln_g"])), r(f32(inputs["ffn_ln_b"]))],
                          axis=2).astype(np.float32)   # [L, 128, 30]
    rows = np.concatenate([f32(inputs["bv"]), f32(inputs["bo"]), wesum],
                          axis=1).astype(np.float32)   # [L, 3*D]

    shared = {
        "wemb": f32(inputs["word_emb"]),
        "pemb": f32(inputs["pos_emb"])[:S],
        "temb": f32(inputs["type_emb"]),
        "eg": f32(inputs["emb_ln_g"]), "eb": f32(inputs["emb_ln_b"]),
        "wq": bf(_pack_dd(WqT)),
        "wk": bf(_pack_dd(WkT)),
        "wo": bf(_pack_dd(WoT)),
        "wf": _pack_dd(WfT).astype(np.float32),
        "wva": bf(wva), "wvb": bf(wvb),
        "cols": np.ascontiguousarray(cols),
        "rows": np.ascontiguousarray(rows),
        "ones128": np.ones(128, np.float32),
        "ones128b": np.ones(128, ml_dtypes.bfloat16),
        "ones512": np.ones((1, 512), np.float32),
        "neg1": np.full((1, 128), -1.0, np.float32),
        "selA": np.concatenate([np.ones((1, 64)), np.zeros((1, 64))], 1).astype(np.float32),
        "selB": np.concatenate([np.zeros((1, 64)), np.ones((1, 64))], 1).astype(np.float32),
        "onesgridb": np.ones((128, NST * H), ml_dtypes.bfloat16),
    }
    ids = np.asarray(inputs["input_ids"]).astype(np.int32)
    tti = np.asarray(inputs["token_type_ids"]).astype(np.int32)
    am = np.asarray(inputs["attention_mask"]).astype(np.float32)
    in_maps = []
    for c in range(B):
        in_maps.append({
            **shared,
            "ids": ids[c].reshape(S, 1),
            "tti": tti[c].reshape(S, 1),
            "maskadd": np.where(am[c] == 0, -1e9, 0.0).astype(np.float32),
        })
    res = bass_utils.run_bass_kernel_spmd(
        nc, in_maps, core_ids=list(range(B)), trace=False)
    out = np.stack([res.results[c]["out"] for c in range(B)], axis=0)
    return out.astype(np.float32)
